# revision 12
# baseline (speedup 1.0000x reference)
"""ConvLSTM (4-layer, 5x5, NH=128) forward + loss on 8 Trainium2 NeuronCores.

Strategy: data-parallel over batch (B=16 -> 2 per core). Each core runs the
full 19-step, 4-layer recurrence on its 2 images. 5x5 SAME convs are computed
on the tensor engine as 25 (or 5, K-packed for the 16-channel frame input)
shifted matmuls accumulating into PSUM: gates[4*NH, 512px] per layer-step.
Gate nonlinearities run on ACT, elementwise combines on DVE, overlapped with
the next accumulation. Weights live resident in SBUF. The scalar loss is
reduced on host from the full predicted sequence (negligible work).
"""
import numpy as np

import concourse.bass as bass
import concourse.mybir as mybir
import concourse.tile as tile
from concourse.vector_clock import ScopedClock

# ---- problem constants (hardcoded per contract) ----
L = 4
NH = 128
FC = 16
HP = WP = 16
B = 16
PRE = 10
AFT = 10
T = PRE + AFT
TS = T - 1          # 19 steps
FS = 5
FORGET_BIAS = 1.0
NCORES = 8
BL = B // NCORES    # 2 images per core
PH, PW = HP + 4, WP + 4
NPX = BL * HP * WP  # 512
G4 = 4 * NH

F32 = mybir.dt.float32
AF = mybir.ActivationFunctionType

# matmul storage dtype for weights + activations ("bf16" or "f32r")
MM_MODE = "bf16"


class _TC(tile.TileContext):
    """TileContext whose tail drain carries at most one sem wait per Drain
    instruction (walrus codegen rejects more)."""

    def _drain_and_barrier(self, tick_clock, wait_clock):
        drain_inst = self.nc.sync.drain()
        wait_clock.add_sem_waits(
            drain_inst.ins, ScopedClock({None: tick_clock.global_clock})
        )
        si = drain_inst.ins.sync_info
        waits = list(si.on_wait) if si and si.on_wait else []
        if len(waits) > 1:
            drain_inst.ins.sync_info = mybir.SyncInfo(
                on_wait=waits[:1], on_update=list(si.on_update)
            )
            for i in range(1, len(waits)):
                extra = self.nc.sync.drain()
                extra.ins.sync_info = mybir.SyncInfo(
                    on_wait=[waits[i]], on_update=[]
                )

        self.nc.all_engine_barrier()
        assert self.sems is not None
        popped = self.nc._tile_sem_poison_stack.pop()
        assert popped is self._sem_poison
        self.nc.clear_and_free_semaphores(list(self.sems.allocated().values()))
        self.nc.all_engine_barrier()


def _split_sync_waits(nc, limit=1):
    """Walrus's per-instruction sync-wait slots are tight (1 for Drain and
    the fp32/f32r matmul LDW lowering). Hoist excess waits onto NoOps."""
    for f in nc.m.functions:
        for bb in f.blocks:
            insts = list(bb.instructions)
            out, changed = [], False
            for inst in insts:
                si = inst.sync_info
                waits = list(si.on_wait) if si and si.on_wait else []
                if len(waits) > limit:
                    changed = True
                    for i in range(0, len(waits) - limit):
                        nop = mybir.InstNoOp(
                            name=f"{inst.name}_w{i}",
                            engine=inst.engine,
                            bass_nofuse=True,
                            sync_info=mybir.SyncInfo(
                                on_wait=[waits[i]], on_update=[]
                            ),
                        )
                        out.append(nop)
                    inst.sync_info = mybir.SyncInfo(
                        on_wait=waits[len(waits) - limit:],
                        on_update=list(si.on_update),
                    )
                out.append(inst)
            if changed:
                bb.instructions = out


def build_nc(mm_mode=MM_MODE):
    dt_mm = mybir.dt.bfloat16 if mm_mode == "bf16" else F32
    use_f32r = mm_mode == "f32r"

    def mmv(ap):
        # view an fp32 AP as float32r for full-rate fp32 matmul
        return ap.bitcast(mybir.dt.float32r) if use_f32r else ap

    nc = bass.Bass()

    fx_d = nc.dram_tensor("fx", [PRE, FC, BL, HP, WP], dt_mm, kind="ExternalInput")
    mf_d = nc.dram_tensor("mf", [AFT - 1, FC, BL, HP, WP], F32, kind="ExternalInput")
    mc_d = nc.dram_tensor("mc", [AFT - 1, FC, BL, HP, WP], F32, kind="ExternalInput")
    wx0_d = nc.dram_tensor("wx0", [FS * FC, FS, G4], dt_mm, kind="ExternalInput")
    wh_d = [nc.dram_tensor(f"wh{l}", [NH, FS * FS, G4], dt_mm, kind="ExternalInput")
            for l in range(L)]
    wx_d = [None] + [nc.dram_tensor(f"wx{l}", [NH, FS * FS, G4], dt_mm,
                                    kind="ExternalInput") for l in range(1, L)]
    wl_d = nc.dram_tensor("wlast", [NH, FC], dt_mm, kind="ExternalInput")
    gen_d = nc.dram_tensor("gen", [TS, FC, BL, HP, WP], F32, kind="ExternalOutput")

    with _TC(nc) as tc:
        with (
            tc.tile_pool(name="singles", bufs=1) as singles,
            tc.tile_pool(name="scratch", bufs=1) as sg,
            tc.tile_pool(name="small", bufs=2) as sm,
            tc.tile_pool(name="wstream", bufs=(2 if mm_mode == "bf16" else 3)) as ws,
            tc.tile_pool(name="gates_ps", bufs=6, space="PSUM") as gp,
            tc.tile_pool(name="xgen_ps", bufs=2, space="PSUM") as xp,
        ):
            # ---- weights: resident where SBUF allows, else streamed ----
            # bf16: everything fits except wh0 (streamed).
            # f32r: only layer-1 weights stay resident; wh0,wh2,wx2,wh3,wx3
            # are streamed per-step per-gate-chunk, double-buffered.
            streamed = {0: ("wh",)} if mm_mode == "bf16" else \
                {0: ("wh",), 2: ("wh", "wx"), 3: ("wh", "wx")}

            def is_streamed(kind, l):
                return kind in streamed.get(l, ())

            wx0_t = singles.tile([FS * FC, FS, G4], dt_mm)
            nc.sync.dma_start(wx0_t[:], wx0_d[:])
            wh_t, wx_t = [None], [None]
            for l in range(1, L):
                if is_streamed("wh", l):
                    wh_t.append(None)
                else:
                    w = singles.tile([NH, FS * FS, G4], dt_mm, name=f"wht{l}")
                    nc.sync.dma_start(w[:], wh_d[l][:])
                    wh_t.append(w)
            for l in range(1, L):
                if is_streamed("wx", l):
                    wx_t.append(None)
                else:
                    w = singles.tile([NH, FS * FS, G4], dt_mm, name=f"wxt{l}")
                    nc.sync.dma_start(w[:], wx_d[l][:])
                    wx_t.append(w)
            wl_t = singles.tile([NH, FC], dt_mm)
            nc.sync.dma_start(wl_t[:], wl_d[:])

            # ---- persistent state ----
            h_pad = [singles.tile([NH, BL, PH, PW], dt_mm, name=f"hpad{l}")
                     for l in range(L)]
            c_st = [singles.tile([NH, BL, HP, WP], F32, name=f"cst{l}")
                    for l in range(L)]
            x_pad = singles.tile([FC, BL, PH, PW], dt_mm)
            xpack = singles.tile([FS * FC, BL, PH, WP], dt_mm)
            for l in range(L):
                nc.vector.memset(h_pad[l][:], 0.0)
                nc.vector.memset(c_st[l][:], 0.0)
            nc.vector.memset(x_pad[:], 0.0)

            def hi(l):  # interior view of h_pad[l]
                return h_pad[l][:, :, 2:2 + HP, 2:2 + WP]

            xgen_prev = None
            for t in range(TS):
                # ---- frame input for this step ----
                if t < PRE:
                    for b in range(BL):
                        nc.sync.dma_start(x_pad[:, b, 2:2 + HP, 2:2 + WP],
                                          fx_d[t, :, b])
                else:
                    mf_t = sm.tile([FC, BL, HP, WP], F32, tag="mf")
                    mc_t = sm.tile([FC, BL, HP, WP], F32, tag="mc")
                    nc.sync.dma_start(mf_t[:], mf_d[t - PRE])
                    nc.sync.dma_start(mc_t[:], mc_d[t - PRE])
                    nc.vector.tensor_mul(mc_t[:], mc_t[:], xgen_prev[:])
                    nc.vector.tensor_add(x_pad[:, :, 2:2 + HP, 2:2 + WP],
                                         mc_t[:], mf_t[:])
                # packed (K=80) frame tile for the layer-0 Wx conv
                for dx in range(FS):
                    nc.sync.dma_start(xpack[FC * dx:FC * (dx + 1)],
                                      x_pad[:, :, :, dx:dx + WP])

                # ---- layers ----
                for l in range(L):
                    gates = [gp.tile([NH, BL, HP, WP], F32, tag="gates",
                                     name=f"g{t}_{l}_{g}") for g in range(4)]
                    def wchunk(kind, l, g):
                        dram = wh_d[l] if kind == "wh" else wx_d[l]
                        res = wh_t[l] if kind == "wh" else wx_t[l]
                        if res is not None:
                            return res[:, :, g * NH:(g + 1) * NH]
                        wc = ws.tile([NH, FS * FS, NH], dt_mm, tag="wstr",
                                     name=f"w{kind}{t}_{l}_{g}")
                        nc.sync.dma_start(wc[:], dram[:, :, g * NH:(g + 1) * NH])
                        return wc[:]

                    # h-recurrence taps first (state from t-1; zero at t=0)
                    if t > 0:
                        for g in range(4):
                            whg = wchunk("wh", l, g)
                            for tap in range(FS * FS):
                                dy, dx = divmod(tap, FS)
                                nc.tensor.matmul(
                                    gates[g][:],
                                    mmv(whg[:, tap, :]),
                                    mmv(h_pad[l][:, :, dy:dy + HP, dx:dx + WP]),
                                    start=(tap == 0), stop=False,
                                )
                    # input taps
                    for g in range(4):
                        if l == 0:
                            for dy in range(FS):
                                nc.tensor.matmul(
                                    gates[g][:],
                                    mmv(wx0_t[:, dy, g * NH:(g + 1) * NH]),
                                    mmv(xpack[:, :, dy:dy + HP, :]),
                                    start=(t == 0 and dy == 0), stop=(dy == FS - 1),
                                )
                        else:
                            wxg = wchunk("wx", l, g)
                            for tap in range(FS * FS):
                                dy, dx = divmod(tap, FS)
                                nc.tensor.matmul(
                                    gates[g][:],
                                    mmv(wxg[:, tap, :]),
                                    mmv(h_pad[l - 1][:, :, dy:dy + HP, dx:dx + WP]),
                                    start=(t == 0 and tap == 0),
                                    stop=(tap == FS * FS - 1),
                                )
                    # gate math: ig fg gg og = gates[0..3]
                    sig = sg.tile([NH, BL, HP, WP], F32, tag="sig")
                    sfg = sg.tile([NH, BL, HP, WP], F32, tag="sfg")
                    tgg = sg.tile([NH, BL, HP, WP], F32, tag="tgg")
                    nc.scalar.activation(sig[:], gates[0][:], AF.Sigmoid)
                    nc.scalar.activation(sfg[:], gates[1][:], AF.Sigmoid,
                                         bias=FORGET_BIAS)
                    nc.scalar.activation(tgg[:], gates[2][:], AF.Tanh)
                    if t > 0:
                        nc.vector.tensor_mul(c_st[l][:], c_st[l][:], sfg[:])
                    nc.vector.tensor_mul(sig[:], sig[:], tgg[:])
                    nc.vector.tensor_add(c_st[l][:], c_st[l][:], sig[:])
                    th = sg.tile([NH, BL, HP, WP], F32, tag="sfg", name="th")
                    sog = sg.tile([NH, BL, HP, WP], F32, tag="tgg", name="sog")
                    nc.scalar.activation(th[:], c_st[l][:], AF.Tanh)
                    nc.scalar.activation(sog[:], gates[3][:], AF.Sigmoid)
                    nc.vector.tensor_mul(hi(l), th[:], sog[:])

                # ---- 1x1 output projection ----
                xg_ps = xp.tile([FC, BL, HP, WP], F32, tag="xg")
                nc.tensor.matmul(xg_ps[:], mmv(wl_t[:]), mmv(hi(L - 1)),
                                 start=True, stop=True)
                xgen = sg.tile([FC, BL, HP, WP], F32, tag="xgen", bufs=2)
                nc.scalar.activation(xgen[:], xg_ps[:], AF.Copy)
                nc.sync.dma_start(gen_d[t], xgen[:])
                xgen_prev = xgen

    _split_sync_waits(nc, limit=1)
    return nc


# ---------------- host side ----------------

_NC_CACHE = {}


def _get_nc(mm_mode=MM_MODE):
    if mm_mode not in _NC_CACHE:
        _NC_CACHE[mm_mode] = build_nc(mm_mode)
    return _NC_CACHE[mm_mode]


def _np_mm_dtype(mm_mode):
    return mybir.dt.np(mybir.dt.bfloat16) if mm_mode == "bf16" else np.float32


def make_in_maps(frames_tensor, mask_true, Wx, Wh, W_last, mm_mode=MM_MODE):
    """Build the per-core input maps (host-side shard + relayout)."""
    npdt = _np_mm_dtype(mm_mode)
    f32 = np.float32

    # weights, shared by all cores
    def lhsT(w):  # [cout, cin, kh, kw] -> [cin, kh*kw, cout]
        cout, cin = w.shape[:2]
        return np.ascontiguousarray(
            w.transpose(1, 2, 3, 0).reshape(cin, FS * FS, cout))

    wmaps = {}
    wx0 = Wx[0].transpose(1, 2, 3, 0)          # [16, 5dy, 5dx, 512]
    wx0 = wx0.transpose(2, 0, 1, 3)            # [5dx, 16, 5dy, 512]
    wmaps["wx0"] = np.ascontiguousarray(
        wx0.reshape(FS * FC, FS, G4)).astype(npdt)
    for l in range(L):
        wmaps[f"wh{l}"] = lhsT(Wh[l]).astype(npdt)
        if l:
            wmaps[f"wx{l}"] = lhsT(Wx[l]).astype(npdt)
    wmaps["wlast"] = np.ascontiguousarray(
        W_last.transpose(1, 0, 2, 3).reshape(NH, FC)).astype(npdt)

    # frames: [B,T,H,W,C] -> per core [t, C, bl, H, W]
    ft = frames_tensor.transpose(1, 4, 0, 2, 3)      # [T, C, B, H, W]
    mt = mask_true.transpose(1, 4, 0, 2, 3)          # [AFT-1, C, B, H, W]

    in_maps = []
    for core in range(NCORES):
        b0 = core * BL
        fx = np.ascontiguousarray(ft[:PRE, :, b0:b0 + BL]).astype(npdt)
        f_late = ft[PRE:TS, :, b0:b0 + BL].astype(f32)   # frames t=10..18
        m = mt[:, :, b0:b0 + BL].astype(f32)
        mf = np.ascontiguousarray(m * f_late)
        mc = np.ascontiguousarray(1.0 - m)
        im = {"fx": fx, "mf": mf, "mc": mc}
        im.update(wmaps)
        in_maps.append(im)
    return in_maps


def assemble_output(results):
    """Per-core gen [TS, FC, BL, HP, WP] -> next_frames [B, TS, HP, WP, FC]."""
    gens = [results[c]["gen"] for c in range(NCORES)]
    g = np.stack(gens)                       # [cores, TS, C, BL, H, W]
    g = g.transpose(0, 3, 1, 4, 5, 2)        # [cores, BL, TS, H, W, C]
    return np.ascontiguousarray(g.reshape(B, TS, HP, WP, FC))


def host_loss(pred, true):
    pred = pred.astype(np.float64)
    true = true.astype(np.float64)
    mse = np.mean((pred - true) ** 2)

    def xi(a):
        p = np.transpose(a[:, :, 0, :, :], (3, 2, 1, 0))  # [C,W,T-1,B]
        p = p - p.max(axis=-1, keepdims=True)
        e = np.exp(p)
        smx = e / e.sum(axis=-1, keepdims=True)
        idx = np.arange(smx.shape[-1], dtype=np.float64)
        return np.mean((smx * idx).sum(-1), axis=0).reshape(-1)

    heuristic = np.mean((xi(pred) - xi(true)) ** 2)
    return np.float32(mse + heuristic)


def kernel(frames_tensor, mask_true, Wx0, Wh0, Wx1, Wh1, Wx2, Wh2, Wx3, Wh3,
           W_last):
    from concourse import bass2jax

    frames_tensor = np.asarray(frames_tensor, np.float32)
    mask_true = np.asarray(mask_true, np.float32)
    Wx = [np.asarray(w, np.float32) for w in (Wx0, Wx1, Wx2, Wx3)]
    Wh = [np.asarray(w, np.float32) for w in (Wh0, Wh1, Wh2, Wh3)]
    W_last = np.asarray(W_last, np.float32)

    nc = _get_nc(MM_MODE)
    in_maps = make_in_maps(frames_tensor, mask_true, Wx, Wh, W_last, MM_MODE)
    results = bass2jax.run_bass_via_pjrt(nc, in_maps, n_cores=NCORES)
    next_frames = assemble_output(results)
    loss = host_loss(next_frames, frames_tensor[:, 1:])
    return next_frames, loss


# revision 17
# speedup vs baseline: 34.5173x; 34.5173x over previous
"""ConvLSTM (4-layer, 5x5, NH=128) forward + loss on 8 Trainium2 NeuronCores.

Strategy: data-parallel over batch (B=16 -> 2 per core). Each core runs the
full 19-step, 4-layer recurrence on its 2 images. 5x5 SAME convs are computed
on the tensor engine as 25 (or 5, K-packed for the 16-channel frame input)
shifted matmuls accumulating into PSUM: gates[4*NH, 512px] per layer-step.
Gate nonlinearities run on ACT, elementwise combines on DVE, overlapped with
the next accumulation. Weights live resident in SBUF. The scalar loss is
reduced on host from the full predicted sequence (negligible work).
"""
import numpy as np

import concourse.bass as bass
import concourse.mybir as mybir
import concourse.tile as tile
from concourse.vector_clock import ScopedClock

# ---- problem constants (hardcoded per contract) ----
L = 4
NH = 128
FC = 16
HP = WP = 16
B = 16
PRE = 10
AFT = 10
T = PRE + AFT
TS = T - 1          # 19 steps
FS = 5
FORGET_BIAS = 1.0
NCORES = 8
BL = B // NCORES    # 2 images per core
PH, PW = HP + 4, WP + 4
NPX = BL * HP * WP  # 512
G4 = 4 * NH

F32 = mybir.dt.float32
AF = mybir.ActivationFunctionType

# matmul storage dtype for weights + activations ("bf16" or "f32r").
# f32r (fp32 storage, FP22 multiply at full PE rate) gives ~4e-4 rel err
# vs ~5e-3 for bf16 at the same simulated kernel time.
MM_MODE = "f32r"


class _TC(tile.TileContext):
    """TileContext whose tail drain carries at most one sem wait per Drain
    instruction (walrus codegen rejects more)."""

    def _drain_and_barrier(self, tick_clock, wait_clock):
        drain_inst = self.nc.sync.drain()
        wait_clock.add_sem_waits(
            drain_inst.ins, ScopedClock({None: tick_clock.global_clock})
        )
        si = drain_inst.ins.sync_info
        waits = list(si.on_wait) if si and si.on_wait else []
        if len(waits) > 1:
            drain_inst.ins.sync_info = mybir.SyncInfo(
                on_wait=waits[:1], on_update=list(si.on_update)
            )
            for i in range(1, len(waits)):
                extra = self.nc.sync.drain()
                extra.ins.sync_info = mybir.SyncInfo(
                    on_wait=[waits[i]], on_update=[]
                )

        self.nc.all_engine_barrier()
        assert self.sems is not None
        popped = self.nc._tile_sem_poison_stack.pop()
        assert popped is self._sem_poison
        self.nc.clear_and_free_semaphores(list(self.sems.allocated().values()))
        self.nc.all_engine_barrier()


def _split_sync_waits(nc, limit=1):
    """Walrus's per-instruction sync-wait slots are tight (1 for Drain and
    the fp32/f32r matmul LDW lowering). Hoist excess waits onto NoOps."""
    for f in nc.m.functions:
        for bb in f.blocks:
            insts = list(bb.instructions)
            out, changed = [], False
            for inst in insts:
                si = inst.sync_info
                waits = list(si.on_wait) if si and si.on_wait else []
                if len(waits) > limit:
                    changed = True
                    for i in range(0, len(waits) - limit):
                        nop = mybir.InstNoOp(
                            name=f"{inst.name}_w{i}",
                            engine=inst.engine,
                            bass_nofuse=True,
                            sync_info=mybir.SyncInfo(
                                on_wait=[waits[i]], on_update=[]
                            ),
                        )
                        out.append(nop)
                    inst.sync_info = mybir.SyncInfo(
                        on_wait=waits[len(waits) - limit:],
                        on_update=list(si.on_update),
                    )
                out.append(inst)
            if changed:
                bb.instructions = out


def build_nc(mm_mode=MM_MODE):
    # float32r = fp32 storage, FP22 multiply at full PE rate; the BIR verifier
    # requires every producer of an f32r matmul operand to round to f32r, so
    # the matmul-feeding tensors are declared float32r end-to-end.
    dt_mm = mybir.dt.bfloat16 if mm_mode == "bf16" else mybir.dt.float32r

    def mmv(ap):
        return ap

    nc = bass.Bass()

    fx_d = nc.dram_tensor("fx", [PRE, FC, BL, HP, WP], dt_mm, kind="ExternalInput")
    mf_d = nc.dram_tensor("mf", [AFT - 1, FC, BL, HP, WP], F32, kind="ExternalInput")
    mc_d = nc.dram_tensor("mc", [AFT - 1, FC, BL, HP, WP], F32, kind="ExternalInput")
    wx0_d = nc.dram_tensor("wx0", [FS * FC, FS, G4], dt_mm, kind="ExternalInput")
    wh_d = [nc.dram_tensor(f"wh{l}", [NH, FS * FS, G4], dt_mm, kind="ExternalInput")
            for l in range(L)]
    wx_d = [None] + [nc.dram_tensor(f"wx{l}", [NH, FS * FS, G4], dt_mm,
                                    kind="ExternalInput") for l in range(1, L)]
    wl_d = nc.dram_tensor("wlast", [NH, FC], dt_mm, kind="ExternalInput")
    gen_d = nc.dram_tensor("gen", [TS, FC, BL, HP, WP], F32, kind="ExternalOutput")

    with _TC(nc) as tc:
        with (
            tc.tile_pool(name="singles", bufs=1) as singles,
            tc.tile_pool(name="scratch", bufs=1) as sg,
            tc.tile_pool(name="small", bufs=2) as sm,
            tc.tile_pool(name="wstream", bufs=(2 if mm_mode == "bf16" else 3)) as ws,
            tc.tile_pool(name="gates_ps", bufs=6, space="PSUM") as gp,
            tc.tile_pool(name="xgen_ps", bufs=2, space="PSUM") as xp,
        ):
            # ---- weights: resident where SBUF allows, else streamed ----
            # bf16: everything fits except wh0 (streamed).
            # f32r: only layer-1 weights stay resident; wh0,wh2,wx2,wh3,wx3
            # are streamed per-step per-gate-chunk, double-buffered.
            streamed = {0: ("wh",)} if mm_mode == "bf16" else \
                {0: ("wh",), 2: ("wh", "wx"), 3: ("wh", "wx")}

            def is_streamed(kind, l):
                return kind in streamed.get(l, ())

            wx0_t = singles.tile([FS * FC, FS, G4], dt_mm)
            nc.sync.dma_start(wx0_t[:], wx0_d[:])
            wh_t, wx_t = [None], [None]
            for l in range(1, L):
                if is_streamed("wh", l):
                    wh_t.append(None)
                else:
                    w = singles.tile([NH, FS * FS, G4], dt_mm, name=f"wht{l}")
                    nc.sync.dma_start(w[:], wh_d[l][:])
                    wh_t.append(w)
            for l in range(1, L):
                if is_streamed("wx", l):
                    wx_t.append(None)
                else:
                    w = singles.tile([NH, FS * FS, G4], dt_mm, name=f"wxt{l}")
                    nc.sync.dma_start(w[:], wx_d[l][:])
                    wx_t.append(w)
            wl_t = singles.tile([NH, FC], dt_mm)
            nc.sync.dma_start(wl_t[:], wl_d[:])

            # ---- persistent state ----
            h_pad = [singles.tile([NH, BL, PH, PW], dt_mm, name=f"hpad{l}")
                     for l in range(L)]
            c_st = [singles.tile([NH, BL, HP, WP], F32, name=f"cst{l}")
                    for l in range(L)]
            x_pad = singles.tile([FC, BL, PH, PW], dt_mm)
            xpack = singles.tile([FS * FC, BL, PH, WP], dt_mm)
            def zero(ap):
                # Memset can't take float32r directly; zero via an f32 view
                if ap.dtype == mybir.dt.float32r:
                    ap = ap.bitcast(F32)
                nc.vector.memset(ap, 0.0)

            for l in range(L):
                zero(h_pad[l][:])
                zero(c_st[l][:])
            zero(x_pad[:])

            def hi(l):  # interior view of h_pad[l]
                return h_pad[l][:, :, 2:2 + HP, 2:2 + WP]

            xgen_prev = None
            for t in range(TS):
                # ---- frame input for this step ----
                if t < PRE:
                    for b in range(BL):
                        nc.sync.dma_start(x_pad[:, b, 2:2 + HP, 2:2 + WP],
                                          fx_d[t, :, b])
                else:
                    mf_t = sm.tile([FC, BL, HP, WP], F32, tag="mf")
                    mc_t = sm.tile([FC, BL, HP, WP], F32, tag="mc")
                    nc.sync.dma_start(mf_t[:], mf_d[t - PRE])
                    nc.sync.dma_start(mc_t[:], mc_d[t - PRE])
                    nc.vector.tensor_mul(mc_t[:], mc_t[:], xgen_prev[:])
                    nc.vector.tensor_add(x_pad[:, :, 2:2 + HP, 2:2 + WP],
                                         mc_t[:], mf_t[:])
                # packed (K=80) frame tile for the layer-0 Wx conv
                for dx in range(FS):
                    nc.sync.dma_start(xpack[FC * dx:FC * (dx + 1)],
                                      x_pad[:, :, :, dx:dx + WP])

                # ---- layers ----
                for l in range(L):
                    gates = [gp.tile([NH, BL, HP, WP], F32, tag="gates",
                                     name=f"g{t}_{l}_{g}") for g in range(4)]
                    def wchunk(kind, l, g):
                        dram = wh_d[l] if kind == "wh" else wx_d[l]
                        res = wh_t[l] if kind == "wh" else wx_t[l]
                        if res is not None:
                            return res[:, :, g * NH:(g + 1) * NH]
                        wc = ws.tile([NH, FS * FS, NH], dt_mm, tag="wstr",
                                     name=f"w{kind}{t}_{l}_{g}")
                        nc.sync.dma_start(wc[:], dram[:, :, g * NH:(g + 1) * NH])
                        return wc[:]

                    # h-recurrence taps first (state from t-1; zero at t=0)
                    if t > 0:
                        for g in range(4):
                            whg = wchunk("wh", l, g)
                            for tap in range(FS * FS):
                                dy, dx = divmod(tap, FS)
                                nc.tensor.matmul(
                                    gates[g][:],
                                    mmv(whg[:, tap, :]),
                                    mmv(h_pad[l][:, :, dy:dy + HP, dx:dx + WP]),
                                    start=(tap == 0), stop=False,
                                )
                    # input taps
                    for g in range(4):
                        if l == 0:
                            for dy in range(FS):
                                nc.tensor.matmul(
                                    gates[g][:],
                                    mmv(wx0_t[:, dy, g * NH:(g + 1) * NH]),
                                    mmv(xpack[:, :, dy:dy + HP, :]),
                                    start=(t == 0 and dy == 0), stop=(dy == FS - 1),
                                )
                        else:
                            wxg = wchunk("wx", l, g)
                            for tap in range(FS * FS):
                                dy, dx = divmod(tap, FS)
                                nc.tensor.matmul(
                                    gates[g][:],
                                    mmv(wxg[:, tap, :]),
                                    mmv(h_pad[l - 1][:, :, dy:dy + HP, dx:dx + WP]),
                                    start=(t == 0 and tap == 0),
                                    stop=(tap == FS * FS - 1),
                                )
                    # gate math: ig fg gg og = gates[0..3]
                    sig = sg.tile([NH, BL, HP, WP], F32, tag="sig")
                    sfg = sg.tile([NH, BL, HP, WP], F32, tag="sfg")
                    tgg = sg.tile([NH, BL, HP, WP], F32, tag="tgg")
                    nc.scalar.activation(sig[:], gates[0][:], AF.Sigmoid)
                    nc.scalar.activation(sfg[:], gates[1][:], AF.Sigmoid,
                                         bias=FORGET_BIAS)
                    nc.scalar.activation(tgg[:], gates[2][:], AF.Tanh)
                    if t > 0:
                        nc.vector.tensor_mul(c_st[l][:], c_st[l][:], sfg[:])
                    nc.vector.tensor_mul(sig[:], sig[:], tgg[:])
                    nc.vector.tensor_add(c_st[l][:], c_st[l][:], sig[:])
                    th = sg.tile([NH, BL, HP, WP], F32, tag="sfg", name="th")
                    sog = sg.tile([NH, BL, HP, WP], F32, tag="tgg", name="sog")
                    nc.scalar.activation(th[:], c_st[l][:], AF.Tanh)
                    nc.scalar.activation(sog[:], gates[3][:], AF.Sigmoid)
                    nc.vector.tensor_mul(hi(l), th[:], sog[:])

                # ---- 1x1 output projection ----
                xg_ps = xp.tile([FC, BL, HP, WP], F32, tag="xg")
                nc.tensor.matmul(xg_ps[:], mmv(wl_t[:]), mmv(hi(L - 1)),
                                 start=True, stop=True)
                xgen = sg.tile([FC, BL, HP, WP], F32, tag="xgen", bufs=2)
                nc.scalar.activation(xgen[:], xg_ps[:], AF.Copy)
                nc.sync.dma_start(gen_d[t], xgen[:])
                xgen_prev = xgen

    _split_sync_waits(nc, limit=1)
    return nc


# ---------------- host side ----------------

_NC_CACHE = {}
_EXEC_CACHE = {}

# inputs identical on every core -> shard_map-replicated (shipped once)
_SHARED_INPUTS = ("wx0", "wh0", "wh1", "wh2", "wh3", "wx1", "wx2", "wx3",
                  "wlast")


def _get_nc(mm_mode=MM_MODE):
    if mm_mode not in _NC_CACHE:
        _NC_CACHE[mm_mode] = build_nc(mm_mode)
    return _NC_CACHE[mm_mode]


def _get_exec(mm_mode=MM_MODE):
    """Memoized jitted SPMD executable. Returns (fn, in_names, out_names,
    out_avals, n_params). fn takes per-core-varying inputs concatenated on
    axis 0 plus full shared (weight) arrays, and returns output arrays
    concatenated on axis 0."""
    if mm_mode in _EXEC_CACHE:
        return _EXEC_CACHE[mm_mode]
    import jax
    from jax.experimental.shard_map import shard_map
    from jax.sharding import Mesh, PartitionSpec
    from concourse.bass2jax import (_bass_exec_p, install_neuronx_cc_hook,
                                    partition_id_tensor)

    install_neuronx_cc_hook()
    nc = _get_nc(mm_mode)
    partition_name = (nc.partition_id_tensor.name
                      if nc.partition_id_tensor else None)
    in_names, out_names, out_avals, zero_shapes = [], [], [], []
    for alloc in nc.m.functions[0].allocations:
        if not isinstance(alloc, mybir.MemoryLocationSet):
            continue
        name = alloc.memorylocations[0].name
        if alloc.kind == "ExternalInput":
            if name != partition_name:
                in_names.append(name)
        elif alloc.kind == "ExternalOutput":
            shape = tuple(alloc.tensor_shape)
            dtype = mybir.dt.np(alloc.dtype)
            out_names.append(name)
            out_avals.append(jax.core.ShapedArray(shape, dtype))
            zero_shapes.append((shape, dtype))
    n_params = len(in_names)
    all_names = in_names + out_names
    if partition_name is not None:
        all_names.append(partition_name)

    def _body(*args):
        operands = list(args)
        if partition_name is not None:
            operands.append(partition_id_tensor())
        outs = _bass_exec_p.bind(
            *operands,
            out_avals=tuple(out_avals),
            in_names=tuple(all_names),
            out_names=tuple(out_names),
            lowering_input_output_aliases=(),
            sim_require_finite=True,
            sim_require_nnan=True,
            nc=nc,
        )
        return tuple(outs)

    devices = jax.devices()[:NCORES]
    mesh = Mesh(np.asarray(devices), ("core",))
    in_specs = tuple(
        PartitionSpec() if n in _SHARED_INPUTS else PartitionSpec("core")
        for n in in_names
    ) + (PartitionSpec("core"),) * len(out_names)
    out_specs = (PartitionSpec("core"),) * len(out_names)
    donate = tuple(range(n_params, n_params + len(out_names)))
    fn = jax.jit(
        shard_map(_body, mesh=mesh, in_specs=in_specs, out_specs=out_specs,
                  check_rep=False),
        donate_argnums=donate, keep_unused=True,
    )
    _EXEC_CACHE[mm_mode] = (fn, in_names, out_names, out_avals, zero_shapes)
    return _EXEC_CACHE[mm_mode]


def run_device(in_maps, mm_mode=MM_MODE):
    """Execute on 8 cores. in_maps: per-core dicts (shared weights must be
    identical objects or equal arrays across cores)."""
    fn, in_names, out_names, out_avals, zero_shapes = _get_exec(mm_mode)
    args = []
    for n in in_names:
        if n in _SHARED_INPUTS:
            args.append(in_maps[0][n])
        else:
            args.append(np.concatenate([in_maps[c][n] for c in range(NCORES)],
                                       axis=0))
    zeros = [np.zeros((NCORES * s[0], *s[1:]), d) for s, d in zero_shapes]
    out_arrs = fn(*args, *zeros)
    results = []
    for c in range(NCORES):
        results.append({
            name: np.asarray(out_arrs[i]).reshape(NCORES, *out_avals[i].shape)[c]
            for i, name in enumerate(out_names)
        })
    return results


def _np_mm_dtype(mm_mode):
    return mybir.dt.np(mybir.dt.bfloat16) if mm_mode == "bf16" else np.float32


def make_in_maps(frames_tensor, mask_true, Wx, Wh, W_last, mm_mode=MM_MODE):
    """Build the per-core input maps (host-side shard + relayout)."""
    npdt = _np_mm_dtype(mm_mode)
    f32 = np.float32

    # weights, shared by all cores
    def lhsT(w):  # [cout, cin, kh, kw] -> [cin, kh*kw, cout]
        cout, cin = w.shape[:2]
        return np.ascontiguousarray(
            w.transpose(1, 2, 3, 0).reshape(cin, FS * FS, cout))

    wmaps = {}
    wx0 = Wx[0].transpose(1, 2, 3, 0)          # [16, 5dy, 5dx, 512]
    wx0 = wx0.transpose(2, 0, 1, 3)            # [5dx, 16, 5dy, 512]
    wmaps["wx0"] = np.ascontiguousarray(
        wx0.reshape(FS * FC, FS, G4)).astype(npdt)
    for l in range(L):
        wmaps[f"wh{l}"] = lhsT(Wh[l]).astype(npdt)
        if l:
            wmaps[f"wx{l}"] = lhsT(Wx[l]).astype(npdt)
    wmaps["wlast"] = np.ascontiguousarray(
        W_last.transpose(1, 0, 2, 3).reshape(NH, FC)).astype(npdt)

    # frames: [B,T,H,W,C] -> per core [t, C, bl, H, W]
    ft = frames_tensor.transpose(1, 4, 0, 2, 3)      # [T, C, B, H, W]
    mt = mask_true.transpose(1, 4, 0, 2, 3)          # [AFT-1, C, B, H, W]

    in_maps = []
    for core in range(NCORES):
        b0 = core * BL
        fx = np.ascontiguousarray(ft[:PRE, :, b0:b0 + BL]).astype(npdt)
        f_late = ft[PRE:TS, :, b0:b0 + BL].astype(f32)   # frames t=10..18
        m = mt[:, :, b0:b0 + BL].astype(f32)
        mf = np.ascontiguousarray(m * f_late)
        mc = np.ascontiguousarray(1.0 - m)
        im = {"fx": fx, "mf": mf, "mc": mc}
        im.update(wmaps)
        in_maps.append(im)
    return in_maps


def assemble_output(results):
    """Per-core gen [TS, FC, BL, HP, WP] -> next_frames [B, TS, HP, WP, FC]."""
    gens = [results[c]["gen"] for c in range(NCORES)]
    g = np.stack(gens)                       # [cores, TS, C, BL, H, W]
    g = g.transpose(0, 3, 1, 4, 5, 2)        # [cores, BL, TS, H, W, C]
    return np.ascontiguousarray(g.reshape(B, TS, HP, WP, FC))


def host_loss(pred, true):
    pred = pred.astype(np.float64)
    true = true.astype(np.float64)
    mse = np.mean((pred - true) ** 2)

    def xi(a):
        p = np.transpose(a[:, :, 0, :, :], (3, 2, 1, 0))  # [C,W,T-1,B]
        p = p - p.max(axis=-1, keepdims=True)
        e = np.exp(p)
        smx = e / e.sum(axis=-1, keepdims=True)
        idx = np.arange(smx.shape[-1], dtype=np.float64)
        return np.mean((smx * idx).sum(-1), axis=0).reshape(-1)

    heuristic = np.mean((xi(pred) - xi(true)) ** 2)
    return np.float32(mse + heuristic)


def kernel(frames_tensor, mask_true, Wx0, Wh0, Wx1, Wh1, Wx2, Wh2, Wx3, Wh3,
           W_last):
    frames_tensor = np.asarray(frames_tensor, np.float32)
    mask_true = np.asarray(mask_true, np.float32)
    Wx = [np.asarray(w, np.float32) for w in (Wx0, Wx1, Wx2, Wx3)]
    Wh = [np.asarray(w, np.float32) for w in (Wh0, Wh1, Wh2, Wh3)]
    W_last = np.asarray(W_last, np.float32)

    in_maps = make_in_maps(frames_tensor, mask_true, Wx, Wh, W_last, MM_MODE)
    results = run_device(in_maps, MM_MODE)
    next_frames = assemble_output(results)
    loss = host_loss(next_frames, frames_tensor[:, 1:])
    return next_frames, loss


# revision 19
# speedup vs baseline: 121.0534x; 3.5070x over previous
"""ConvLSTM (4-layer, 5x5, NH=128) forward + loss on 8 Trainium2 NeuronCores.

Strategy: data-parallel over batch (B=16 -> 2 per core). Each core runs the
full 19-step, 4-layer recurrence on its 2 images. 5x5 SAME convs are computed
on the tensor engine as 25 (or 5, K-packed for the 16-channel frame input)
shifted matmuls accumulating into PSUM: gates[4*NH, 512px] per layer-step.
Gate nonlinearities run on ACT, elementwise combines on DVE, overlapped with
the next accumulation. Weights live resident in SBUF. The scalar loss is
reduced on host from the full predicted sequence (negligible work).
"""
import numpy as np

import concourse.bass as bass
import concourse.mybir as mybir
import concourse.tile as tile
from concourse.vector_clock import ScopedClock

# ---- problem constants (hardcoded per contract) ----
L = 4
NH = 128
FC = 16
HP = WP = 16
B = 16
PRE = 10
AFT = 10
T = PRE + AFT
TS = T - 1          # 19 steps
FS = 5
FORGET_BIAS = 1.0
NCORES = 8
BL = B // NCORES    # 2 images per core
PH, PW = HP + 4, WP + 4
NPX = BL * HP * WP  # 512
G4 = 4 * NH

F32 = mybir.dt.float32
AF = mybir.ActivationFunctionType

# matmul storage dtype for weights + activations ("bf16" or "f32r").
# f32r (fp32 storage, FP22 multiply at full PE rate) gives ~4e-4 rel err
# vs ~5e-3 for bf16 at the same simulated kernel time.
MM_MODE = "f32r"


class _TC(tile.TileContext):
    """TileContext whose tail drain carries at most one sem wait per Drain
    instruction (walrus codegen rejects more)."""

    def _drain_and_barrier(self, tick_clock, wait_clock):
        drain_inst = self.nc.sync.drain()
        wait_clock.add_sem_waits(
            drain_inst.ins, ScopedClock({None: tick_clock.global_clock})
        )
        si = drain_inst.ins.sync_info
        waits = list(si.on_wait) if si and si.on_wait else []
        if len(waits) > 1:
            drain_inst.ins.sync_info = mybir.SyncInfo(
                on_wait=waits[:1], on_update=list(si.on_update)
            )
            for i in range(1, len(waits)):
                extra = self.nc.sync.drain()
                extra.ins.sync_info = mybir.SyncInfo(
                    on_wait=[waits[i]], on_update=[]
                )

        self.nc.all_engine_barrier()
        assert self.sems is not None
        popped = self.nc._tile_sem_poison_stack.pop()
        assert popped is self._sem_poison
        self.nc.clear_and_free_semaphores(list(self.sems.allocated().values()))
        self.nc.all_engine_barrier()


def _split_sync_waits(nc, limit=1):
    """Walrus's per-instruction sync-wait slots are tight (1 for Drain and
    the fp32/f32r matmul LDW lowering). Hoist excess waits onto NoOps."""
    for f in nc.m.functions:
        for bb in f.blocks:
            insts = list(bb.instructions)
            out, changed = [], False
            for inst in insts:
                si = inst.sync_info
                waits = list(si.on_wait) if si and si.on_wait else []
                if len(waits) > limit:
                    changed = True
                    for i in range(0, len(waits) - limit):
                        nop = mybir.InstNoOp(
                            name=f"{inst.name}_w{i}",
                            engine=inst.engine,
                            bass_nofuse=True,
                            sync_info=mybir.SyncInfo(
                                on_wait=[waits[i]], on_update=[]
                            ),
                        )
                        out.append(nop)
                    inst.sync_info = mybir.SyncInfo(
                        on_wait=waits[len(waits) - limit:],
                        on_update=list(si.on_update),
                    )
                out.append(inst)
            if changed:
                bb.instructions = out


def build_nc(mm_mode=MM_MODE):
    # float32r = fp32 storage, FP22 multiply at full PE rate; the BIR verifier
    # requires every producer of an f32r matmul operand to round to f32r, so
    # the matmul-feeding tensors are declared float32r end-to-end.
    dt_mm = mybir.dt.bfloat16 if mm_mode == "bf16" else mybir.dt.float32r

    def mmv(ap):
        return ap

    nc = bass.Bass()

    fx_d = nc.dram_tensor("fx", [PRE, FC, BL, HP, WP], dt_mm, kind="ExternalInput")
    mf_d = nc.dram_tensor("mf", [AFT - 1, FC, BL, HP, WP], F32, kind="ExternalInput")
    mc_d = nc.dram_tensor("mc", [AFT - 1, FC, BL, HP, WP], F32, kind="ExternalInput")
    wx0_d = nc.dram_tensor("wx0", [FS * FC, FS, G4], dt_mm, kind="ExternalInput")
    wh_d = [nc.dram_tensor(f"wh{l}", [NH, FS * FS, G4], dt_mm, kind="ExternalInput")
            for l in range(L)]
    wx_d = [None] + [nc.dram_tensor(f"wx{l}", [NH, FS * FS, G4], dt_mm,
                                    kind="ExternalInput") for l in range(1, L)]
    wl_d = nc.dram_tensor("wlast", [NH, FC], dt_mm, kind="ExternalInput")
    gen_d = nc.dram_tensor("gen", [TS, FC, BL, HP, WP], F32, kind="ExternalOutput")

    with _TC(nc) as tc:
        with (
            tc.tile_pool(name="singles", bufs=1) as singles,
            tc.tile_pool(name="scratch", bufs=1) as sg,
            tc.tile_pool(name="small", bufs=2) as sm,
            tc.tile_pool(name="wstream", bufs=(2 if mm_mode == "bf16" else 3)) as ws,
            tc.tile_pool(name="gates_ps", bufs=6, space="PSUM") as gp,
            tc.tile_pool(name="xgen_ps", bufs=2, space="PSUM") as xp,
        ):
            # ---- weights: resident where SBUF allows, else streamed ----
            # bf16: everything fits except wh0 (streamed).
            # f32r: only layer-1 weights stay resident; wh0,wh2,wx2,wh3,wx3
            # are streamed per-step per-gate-chunk, double-buffered.
            streamed = {0: ("wh",)} if mm_mode == "bf16" else \
                {0: ("wh",), 2: ("wh", "wx"), 3: ("wh", "wx")}

            def is_streamed(kind, l):
                return kind in streamed.get(l, ())

            wx0_t = singles.tile([FS * FC, FS, G4], dt_mm)
            nc.sync.dma_start(wx0_t[:], wx0_d[:])
            wh_t, wx_t = [None], [None]
            for l in range(1, L):
                if is_streamed("wh", l):
                    wh_t.append(None)
                else:
                    w = singles.tile([NH, FS * FS, G4], dt_mm, name=f"wht{l}")
                    nc.sync.dma_start(w[:], wh_d[l][:])
                    wh_t.append(w)
            for l in range(1, L):
                if is_streamed("wx", l):
                    wx_t.append(None)
                else:
                    w = singles.tile([NH, FS * FS, G4], dt_mm, name=f"wxt{l}")
                    nc.sync.dma_start(w[:], wx_d[l][:])
                    wx_t.append(w)
            wl_t = singles.tile([NH, FC], dt_mm)
            nc.sync.dma_start(wl_t[:], wl_d[:])

            # ---- persistent state ----
            h_pad = [singles.tile([NH, BL, PH, PW], dt_mm, name=f"hpad{l}")
                     for l in range(L)]
            c_st = [singles.tile([NH, BL, HP, WP], F32, name=f"cst{l}")
                    for l in range(L)]
            x_pad = singles.tile([FC, BL, PH, PW], dt_mm)
            xpack = singles.tile([FS * FC, BL, PH, WP], dt_mm)
            def zero(ap):
                # Memset can't take float32r directly; zero via an f32 view
                if ap.dtype == mybir.dt.float32r:
                    ap = ap.bitcast(F32)
                nc.vector.memset(ap, 0.0)

            for l in range(L):
                zero(h_pad[l][:])
                zero(c_st[l][:])
            zero(x_pad[:])

            def hi(l):  # interior view of h_pad[l]
                return h_pad[l][:, :, 2:2 + HP, 2:2 + WP]

            xgen_prev = None
            for t in range(TS):
                # ---- frame input for this step ----
                if t < PRE:
                    for b in range(BL):
                        nc.sync.dma_start(x_pad[:, b, 2:2 + HP, 2:2 + WP],
                                          fx_d[t, :, b])
                else:
                    mf_t = sm.tile([FC, BL, HP, WP], F32, tag="mf")
                    mc_t = sm.tile([FC, BL, HP, WP], F32, tag="mc")
                    nc.sync.dma_start(mf_t[:], mf_d[t - PRE])
                    nc.sync.dma_start(mc_t[:], mc_d[t - PRE])
                    nc.vector.tensor_mul(mc_t[:], mc_t[:], xgen_prev[:])
                    nc.vector.tensor_add(x_pad[:, :, 2:2 + HP, 2:2 + WP],
                                         mc_t[:], mf_t[:])
                # packed (K=80) frame tile for the layer-0 Wx conv
                for dx in range(FS):
                    nc.sync.dma_start(xpack[FC * dx:FC * (dx + 1)],
                                      x_pad[:, :, :, dx:dx + WP])

                # ---- layers ----
                for l in range(L):
                    gates = [gp.tile([NH, BL, HP, WP], F32, tag="gates",
                                     name=f"g{t}_{l}_{g}") for g in range(4)]
                    def wchunk(kind, l, g):
                        dram = wh_d[l] if kind == "wh" else wx_d[l]
                        res = wh_t[l] if kind == "wh" else wx_t[l]
                        if res is not None:
                            return res[:, :, g * NH:(g + 1) * NH]
                        wc = ws.tile([NH, FS * FS, NH], dt_mm, tag="wstr",
                                     name=f"w{kind}{t}_{l}_{g}")
                        nc.sync.dma_start(wc[:], dram[:, :, g * NH:(g + 1) * NH])
                        return wc[:]

                    # h-recurrence taps first (state from t-1; zero at t=0)
                    if t > 0:
                        for g in range(4):
                            whg = wchunk("wh", l, g)
                            for tap in range(FS * FS):
                                dy, dx = divmod(tap, FS)
                                nc.tensor.matmul(
                                    gates[g][:],
                                    mmv(whg[:, tap, :]),
                                    mmv(h_pad[l][:, :, dy:dy + HP, dx:dx + WP]),
                                    start=(tap == 0), stop=False,
                                )
                    # input taps
                    for g in range(4):
                        if l == 0:
                            for dy in range(FS):
                                nc.tensor.matmul(
                                    gates[g][:],
                                    mmv(wx0_t[:, dy, g * NH:(g + 1) * NH]),
                                    mmv(xpack[:, :, dy:dy + HP, :]),
                                    start=(t == 0 and dy == 0), stop=(dy == FS - 1),
                                )
                        else:
                            wxg = wchunk("wx", l, g)
                            for tap in range(FS * FS):
                                dy, dx = divmod(tap, FS)
                                nc.tensor.matmul(
                                    gates[g][:],
                                    mmv(wxg[:, tap, :]),
                                    mmv(h_pad[l - 1][:, :, dy:dy + HP, dx:dx + WP]),
                                    start=(t == 0 and tap == 0),
                                    stop=(tap == FS * FS - 1),
                                )
                    # gate math: ig fg gg og = gates[0..3]
                    sig = sg.tile([NH, BL, HP, WP], F32, tag="sig")
                    sfg = sg.tile([NH, BL, HP, WP], F32, tag="sfg")
                    tgg = sg.tile([NH, BL, HP, WP], F32, tag="tgg")
                    nc.scalar.activation(sig[:], gates[0][:], AF.Sigmoid)
                    nc.scalar.activation(sfg[:], gates[1][:], AF.Sigmoid,
                                         bias=FORGET_BIAS)
                    nc.scalar.activation(tgg[:], gates[2][:], AF.Tanh)
                    if t > 0:
                        nc.vector.tensor_mul(c_st[l][:], c_st[l][:], sfg[:])
                    nc.vector.tensor_mul(sig[:], sig[:], tgg[:])
                    nc.vector.tensor_add(c_st[l][:], c_st[l][:], sig[:])
                    th = sg.tile([NH, BL, HP, WP], F32, tag="sfg", name="th")
                    sog = sg.tile([NH, BL, HP, WP], F32, tag="tgg", name="sog")
                    nc.scalar.activation(th[:], c_st[l][:], AF.Tanh)
                    nc.scalar.activation(sog[:], gates[3][:], AF.Sigmoid)
                    nc.vector.tensor_mul(hi(l), th[:], sog[:])

                # ---- 1x1 output projection ----
                xg_ps = xp.tile([FC, BL, HP, WP], F32, tag="xg")
                nc.tensor.matmul(xg_ps[:], mmv(wl_t[:]), mmv(hi(L - 1)),
                                 start=True, stop=True)
                xgen = sg.tile([FC, BL, HP, WP], F32, tag="xgen", bufs=2)
                nc.scalar.activation(xgen[:], xg_ps[:], AF.Copy)
                nc.sync.dma_start(gen_d[t], xgen[:])
                xgen_prev = xgen

    _split_sync_waits(nc, limit=1)
    return nc


# ---------------- host side ----------------

_NC_CACHE = {}
_EXEC_CACHE = {}

# inputs identical on every core -> shard_map-replicated (shipped once)
_SHARED_INPUTS = ("wx0", "wh0", "wh1", "wh2", "wh3", "wx1", "wx2", "wx3",
                  "wlast")


def _get_nc(mm_mode=MM_MODE):
    if mm_mode not in _NC_CACHE:
        _NC_CACHE[mm_mode] = build_nc(mm_mode)
    return _NC_CACHE[mm_mode]


def _get_exec(mm_mode=MM_MODE, donate=True):
    """Memoized jitted SPMD executable. Returns (fn, in_names, out_names,
    out_avals, zero_shapes). fn takes per-core-varying inputs concatenated on
    axis 0 plus full shared (weight) arrays, and returns output arrays
    concatenated on axis 0. donate=False lets callers reuse device-resident
    zero output buffers across calls (the kernel writes every output element,
    so pre-zeroed contents are irrelevant)."""
    key = (mm_mode, donate)
    if key in _EXEC_CACHE:
        return _EXEC_CACHE[key]
    import jax
    from jax.experimental.shard_map import shard_map
    from jax.sharding import Mesh, PartitionSpec
    from concourse.bass2jax import (_bass_exec_p, install_neuronx_cc_hook,
                                    partition_id_tensor)

    install_neuronx_cc_hook()
    nc = _get_nc(mm_mode)
    partition_name = (nc.partition_id_tensor.name
                      if nc.partition_id_tensor else None)
    in_names, out_names, out_avals, zero_shapes = [], [], [], []
    for alloc in nc.m.functions[0].allocations:
        if not isinstance(alloc, mybir.MemoryLocationSet):
            continue
        name = alloc.memorylocations[0].name
        if alloc.kind == "ExternalInput":
            if name != partition_name:
                in_names.append(name)
        elif alloc.kind == "ExternalOutput":
            shape = tuple(alloc.tensor_shape)
            dtype = mybir.dt.np(alloc.dtype)
            out_names.append(name)
            out_avals.append(jax.core.ShapedArray(shape, dtype))
            zero_shapes.append((shape, dtype))
    n_params = len(in_names)
    all_names = in_names + out_names
    if partition_name is not None:
        all_names.append(partition_name)

    def _body(*args):
        operands = list(args)
        if partition_name is not None:
            operands.append(partition_id_tensor())
        outs = _bass_exec_p.bind(
            *operands,
            out_avals=tuple(out_avals),
            in_names=tuple(all_names),
            out_names=tuple(out_names),
            lowering_input_output_aliases=(),
            sim_require_finite=True,
            sim_require_nnan=True,
            nc=nc,
        )
        return tuple(outs)

    devices = jax.devices()[:NCORES]
    mesh = Mesh(np.asarray(devices), ("core",))
    in_specs = tuple(
        PartitionSpec() if n in _SHARED_INPUTS else PartitionSpec("core")
        for n in in_names
    ) + (PartitionSpec("core"),) * len(out_names)
    out_specs = (PartitionSpec("core"),) * len(out_names)
    donate_idx = tuple(range(n_params, n_params + len(out_names))) if donate \
        else ()
    fn = jax.jit(
        shard_map(_body, mesh=mesh, in_specs=in_specs, out_specs=out_specs,
                  check_rep=False),
        donate_argnums=donate_idx, keep_unused=True,
    )
    _EXEC_CACHE[key] = (fn, in_names, out_names, out_avals, zero_shapes)
    return _EXEC_CACHE[key]


def run_device(in_maps, mm_mode=MM_MODE):
    """Execute on 8 cores. in_maps: per-core dicts (shared weights must be
    identical objects or equal arrays across cores)."""
    fn, in_names, out_names, out_avals, zero_shapes = _get_exec(mm_mode)
    args = []
    for n in in_names:
        if n in _SHARED_INPUTS:
            args.append(in_maps[0][n])
        else:
            args.append(np.concatenate([in_maps[c][n] for c in range(NCORES)],
                                       axis=0))
    zeros = [np.zeros((NCORES * s[0], *s[1:]), d) for s, d in zero_shapes]
    out_arrs = fn(*args, *zeros)
    results = []
    for c in range(NCORES):
        results.append({
            name: np.asarray(out_arrs[i]).reshape(NCORES, *out_avals[i].shape)[c]
            for i, name in enumerate(out_names)
        })
    return results


def _np_mm_dtype(mm_mode):
    return mybir.dt.np(mybir.dt.bfloat16) if mm_mode == "bf16" else np.float32


def make_in_maps(frames_tensor, mask_true, Wx, Wh, W_last, mm_mode=MM_MODE):
    """Build the per-core input maps (host-side shard + relayout)."""
    npdt = _np_mm_dtype(mm_mode)
    f32 = np.float32

    # weights, shared by all cores
    def lhsT(w):  # [cout, cin, kh, kw] -> [cin, kh*kw, cout]
        cout, cin = w.shape[:2]
        return np.ascontiguousarray(
            w.transpose(1, 2, 3, 0).reshape(cin, FS * FS, cout))

    wmaps = {}
    wx0 = Wx[0].transpose(1, 2, 3, 0)          # [16, 5dy, 5dx, 512]
    wx0 = wx0.transpose(2, 0, 1, 3)            # [5dx, 16, 5dy, 512]
    wmaps["wx0"] = np.ascontiguousarray(
        wx0.reshape(FS * FC, FS, G4)).astype(npdt)
    for l in range(L):
        wmaps[f"wh{l}"] = lhsT(Wh[l]).astype(npdt)
        if l:
            wmaps[f"wx{l}"] = lhsT(Wx[l]).astype(npdt)
    wmaps["wlast"] = np.ascontiguousarray(
        W_last.transpose(1, 0, 2, 3).reshape(NH, FC)).astype(npdt)

    # frames: [B,T,H,W,C] -> per core [t, C, bl, H, W]
    ft = frames_tensor.transpose(1, 4, 0, 2, 3)      # [T, C, B, H, W]
    mt = mask_true.transpose(1, 4, 0, 2, 3)          # [AFT-1, C, B, H, W]

    in_maps = []
    for core in range(NCORES):
        b0 = core * BL
        fx = np.ascontiguousarray(ft[:PRE, :, b0:b0 + BL]).astype(npdt)
        f_late = ft[PRE:TS, :, b0:b0 + BL].astype(f32)   # frames t=10..18
        m = mt[:, :, b0:b0 + BL].astype(f32)
        mf = np.ascontiguousarray(m * f_late)
        mc = np.ascontiguousarray(1.0 - m)
        im = {"fx": fx, "mf": mf, "mc": mc}
        im.update(wmaps)
        in_maps.append(im)
    return in_maps


def assemble_output(results):
    """Per-core gen [TS, FC, BL, HP, WP] -> next_frames [B, TS, HP, WP, FC]."""
    gens = [results[c]["gen"] for c in range(NCORES)]
    g = np.stack(gens)                       # [cores, TS, C, BL, H, W]
    g = g.transpose(0, 3, 1, 4, 5, 2)        # [cores, BL, TS, H, W, C]
    return np.ascontiguousarray(g.reshape(B, TS, HP, WP, FC))


def host_loss(pred, true):
    pred = pred.astype(np.float64)
    true = true.astype(np.float64)
    mse = np.mean((pred - true) ** 2)

    def xi(a):
        p = np.transpose(a[:, :, 0, :, :], (3, 2, 1, 0))  # [C,W,T-1,B]
        p = p - p.max(axis=-1, keepdims=True)
        e = np.exp(p)
        smx = e / e.sum(axis=-1, keepdims=True)
        idx = np.arange(smx.shape[-1], dtype=np.float64)
        return np.mean((smx * idx).sum(-1), axis=0).reshape(-1)

    heuristic = np.mean((xi(pred) - xi(true)) ** 2)
    return np.float32(mse + heuristic)


def kernel(frames_tensor, mask_true, Wx0, Wh0, Wx1, Wh1, Wx2, Wh2, Wx3, Wh3,
           W_last):
    frames_tensor = np.asarray(frames_tensor, np.float32)
    mask_true = np.asarray(mask_true, np.float32)
    Wx = [np.asarray(w, np.float32) for w in (Wx0, Wx1, Wx2, Wx3)]
    Wh = [np.asarray(w, np.float32) for w in (Wh0, Wh1, Wh2, Wh3)]
    W_last = np.asarray(W_last, np.float32)

    in_maps = make_in_maps(frames_tensor, mask_true, Wx, Wh, W_last, MM_MODE)
    results = run_device(in_maps, MM_MODE)
    next_frames = assemble_output(results)
    loss = host_loss(next_frames, frames_tensor[:, 1:])
    return next_frames, loss
